# revision 17
# baseline (speedup 1.0000x reference)
"""Trainium2 kernel for nn_GravityHypothesisTester.

Heavy part (B x N x N distance matrices + row/col min/argmin) runs on 8
NeuronCores: core c handles (batch = c % 4, orientation = c // 4).
Orientation 0 reduces over tgt (rows = src points), orientation 1 reduces
over src (rows = tgt points). The PE computes G[n,m] = 2*s.t - yy[m] via an
augmented matmul so min_m dist2[n,m] = xx[n] - max_m G[n,m] and
argmin = argmax.

v2 layout (vs the fp32 baseline):
- The fp32 operands are split hi/mid/lo into bf16 triples on the host and the
  products expanded into K=21 bf16 rows (error ~2e-6, comparable to fp32
  rounding) so the PE runs at 1 cycle/row instead of fp32's 4.
- ACT copies each PSUM half-tile to SBUF; the DVE max pass then runs
  SBUF->SBUF which enables the DVE 2x_2p perf mode (2 elem/cycle on fp32).
- The argmax-recovery pass (scalar_tensor_tensor is_equal*iota, sum-accum,
  1x-only) is split between DVE and GpSimd to balance engine time.
Host does the tiny O(B*N) pre/post work (Rodrigues, means, median, sigmoid).
"""

import sys
from contextlib import ExitStack

import numpy as np
import ml_dtypes

sys.path.insert(0, "/opt/trn_rl_repo")

import concourse.bass as bass
import concourse.tile as tile
from concourse import bacc, mybir
from concourse.bass_utils import run_bass_kernel_spmd  # noqa: F401

EPS = 1e-6
CHI2_THRESH = 9.0
DIST_SCALE = 3.0
B = 4
N = 4096
P = 128
ROW_TILES = N // P          # 32
HALF = 2048                 # half-tile free size (4 PSUM banks)
MMCOLS = 512                # PSUM-bank-limited moving size (512 fp32 out)
N_CORES = 8
NHALF = ROW_TILES * 2       # 64
KROWS = 21                  # bf16-split product rows (18 coord terms + 3 yy)
BF16 = ml_dtypes.bfloat16

WIN = 128                   # candidate-window width reported to the host
SEGS = N // WIN             # 32 windows per row
SEGS_H = HALF // WIN        # 16 windows per half-tile

_NC_CACHE = {}


def _build_nc(repeat=1):
    """Build the SPMD bass program (identical on all 8 cores)."""
    key = ("nc", repeat)
    if key in _NC_CACHE:
        return _NC_CACHE[key]

    nc = bacc.Bacc("TRN2", target_bir_lowering=False)
    f32 = mybir.dt.float32
    bf16 = mybir.dt.bfloat16

    lhsT_d = nc.dram_tensor("lhsT", [KROWS, N], bf16, kind="ExternalInput")
    rhs_d = nc.dram_tensor("rhs", [KROWS, N], bf16, kind="ExternalInput")
    vals_d = nc.dram_tensor("vals", [P, ROW_TILES], f32, kind="ExternalOutput")
    segs_d = nc.dram_tensor("segs", [P, ROW_TILES], f32, kind="ExternalOutput")

    with tile.TileContext(nc) as tc, ExitStack() as ctx:
        inp = ctx.enter_context(tc.tile_pool(name="inp", bufs=1))
        stage = ctx.enter_context(tc.tile_pool(name="stage", bufs=1))
        cpool = ctx.enter_context(tc.tile_pool(name="cpool", bufs=4))
        spool = ctx.enter_context(tc.tile_pool(name="spool", bufs=3))
        psum_a = ctx.enter_context(tc.tile_pool(name="psum_a", bufs=2, space="PSUM"))

        lhsT_s = inp.tile([KROWS, N], bf16)
        rhs_s = inp.tile([KROWS, N], bf16)
        nc.sync.dma_start(lhsT_s[:], lhsT_d[:])
        nc.sync.dma_start(rhs_s[:], rhs_d[:])

        vstage = stage.tile([P, ROW_TILES], f32)
        sstage = stage.tile([P, ROW_TILES], f32)
        scr = stage.tile([P, SEGS], f32)
        iota_i = stage.tile([P, SEGS], mybir.dt.int32)
        iota_f = stage.tile([P, SEGS], f32)
        nc.gpsimd.iota(iota_i[:], pattern=[[1, SEGS]], base=0,
                       channel_multiplier=0)
        nc.vector.tensor_copy(iota_f[:], iota_i[:])

        # per-row-tile mini ops, issued one row-tile behind the segmax pass and
        # split around a segmax so the accumulator-readback stall (v1 -> s*)
        # hides under the intervening 2048-element reduce.
        def mini_v1(seg_tile, rt):
            nc.vector.tensor_scalar(
                scr[:], seg_tile[:], 0.0, None,
                op0=mybir.AluOpType.add, op1=mybir.AluOpType.max,
                accum_out=vstage[:, rt : rt + 1],
            )

        def mini_s1(seg_tile, rt):
            nc.vector.scalar_tensor_tensor(
                scr[:], seg_tile[:], vstage[:, rt : rt + 1], iota_f[:],
                op0=mybir.AluOpType.is_equal, op1=mybir.AluOpType.mult,
                accum_out=sstage[:, rt : rt + 1],
            )

        prev = None  # (seg_tile, rt) pending minis
        for t in range(ROW_TILES * repeat):
            rt = t % ROW_TILES
            seg_tile = spool.tile([P, SEGS], f32)
            for h in range(2):
                pt = psum_a.tile([P, HALF], f32)
                if t == 0 and h == 0:
                    # Dummy matmul reading only rhs_s: the PE weight-load HW
                    # slot carries a single semaphore wait, so the two input
                    # DMA waits must land on separate PE instructions.
                    nc.tensor.matmul(
                        pt[:, 0:MMCOLS], rhs_s[:, 0:P], rhs_s[:, 0:MMCOLS],
                        start=True, stop=True,
                    )
                for j in range(HALF // MMCOLS):
                    nc.tensor.matmul(
                        pt[:, bass.ts(j, MMCOLS)],
                        lhsT_s[:, bass.ts(rt, P)],
                        rhs_s[:, h * HALF + j * MMCOLS : h * HALF + (j + 1) * MMCOLS],
                        start=True,
                        stop=True,
                    )
                # ACT absorbs the PSUM read; the DVE window-max pass then runs
                # SBUF->SBUF which gets the 2x perf mode.
                sb = cpool.tile([P, HALF], f32)
                nc.scalar.copy(sb[:], pt[:])
                nc.vector.tensor_reduce(
                    seg_tile[:, h * SEGS_H : (h + 1) * SEGS_H],
                    sb[:].rearrange("p (s x) -> p s x", s=SEGS_H),
                    axis=mybir.AxisListType.X,
                    op=mybir.AluOpType.max,
                )
                if prev is not None:
                    if h == 0:
                        mini_v1(*prev)
                    else:
                        mini_s1(*prev)
            prev = (seg_tile, rt)
        mini_v1(*prev)
        mini_s1(*prev)

        nc.sync.dma_start(vals_d[:], vstage[:])
        nc.sync.dma_start(segs_d[:], sstage[:])

    nc.finalize()
    _NC_CACHE[key] = nc
    return nc


def _get_runner(repeat=1):
    """Build the sharded PJRT executable once; reuse across kernel() calls."""
    rkey = ("runner", repeat)
    if rkey in _NC_CACHE:
        return _NC_CACHE[rkey]

    import jax
    from jax.sharding import Mesh, PartitionSpec
    from jax.experimental.shard_map import shard_map
    from concourse import bass2jax

    nc = _build_nc(repeat)
    bass2jax.install_neuronx_cc_hook()

    partition_name = nc.partition_id_tensor.name if nc.partition_id_tensor else None
    in_names, out_names, out_avals, zero_outs = [], [], [], []
    for alloc in nc.m.functions[0].allocations:
        if not isinstance(alloc, mybir.MemoryLocationSet):
            continue
        name = alloc.memorylocations[0].name
        if alloc.kind == "ExternalInput":
            if name != partition_name:
                in_names.append(name)
        elif alloc.kind == "ExternalOutput":
            shape = tuple(alloc.tensor_shape)
            np_dt = mybir.dt.np(alloc.dtype)
            out_names.append(name)
            out_avals.append(jax.core.ShapedArray(shape, np_dt))
            zero_outs.append(np.zeros(shape, np_dt))

    n_params = len(in_names)
    n_outs = len(out_names)
    all_in_names = list(in_names) + list(out_names)
    if partition_name is not None:
        all_in_names.append(partition_name)
    donate = tuple(range(n_params, n_params + n_outs))

    def _body(*args):
        operands = list(args)
        if partition_name is not None:
            operands.append(bass2jax.partition_id_tensor())
        outs = bass2jax._bass_exec_p.bind(
            *operands,
            out_avals=tuple(out_avals),
            in_names=tuple(all_in_names),
            out_names=tuple(out_names),
            lowering_input_output_aliases=(),
            sim_require_finite=True,
            sim_require_nnan=True,
            nc=nc,
        )
        return tuple(outs)

    devices = jax.devices()[:N_CORES]
    mesh = Mesh(np.asarray(devices), ("core",))
    in_specs = (PartitionSpec("core"),) * (n_params + n_outs)
    out_specs = (PartitionSpec("core"),) * n_outs
    sharded = jax.jit(
        shard_map(_body, mesh=mesh, in_specs=in_specs, out_specs=out_specs,
                  check_rep=False),
        donate_argnums=donate, keep_unused=True,
    )

    def run(in_maps):
        concat_in = [
            np.concatenate([np.asarray(m[name]) for m in in_maps], axis=0)
            for name in in_names
        ]
        concat_zeros = [
            np.zeros((N_CORES * z.shape[0], *z.shape[1:]), z.dtype)
            for z in zero_outs
        ]
        out_arrs = sharded(*concat_in, *concat_zeros)
        return [
            {
                name: np.asarray(out_arrs[k]).reshape(
                    N_CORES, *out_avals[k].shape)[c]
                for k, name in enumerate(out_names)
            }
            for c in range(N_CORES)
        ]

    run.sharded = sharded
    run.mesh = mesh
    run.in_names = in_names
    run.zero_outs = zero_outs
    _NC_CACHE[rkey] = run
    return run


# ---------------- host-side numpy port of the tiny reference pieces ----------


def _normalize(x, axis, eps=EPS):
    n = np.linalg.norm(x, axis=axis, keepdims=True)
    return x / np.maximum(n, eps)


def _skew(k):
    kx, ky, kz = k[:, 0], k[:, 1], k[:, 2]
    O = np.zeros_like(kx)
    row0 = np.stack([O, -kz, ky], axis=1)
    row1 = np.stack([kz, O, -kx], axis=1)
    row2 = np.stack([-ky, kx, O], axis=1)
    return np.stack([row0, row1, row2], axis=1)


def _gravity_align(g_src, g_tgt, eps=EPS):
    u = _normalize(g_src, 1, eps)
    v = _normalize(g_tgt, 1, eps)
    axis = np.cross(u, v)
    axis_norm = np.linalg.norm(axis, axis=1, keepdims=True)
    dot = np.clip(np.sum(u * v, axis=1, keepdims=True), -1.0, 1.0)
    parallel = axis_norm < 1e-6
    k = axis / (axis_norm + eps)
    theta = np.arccos(dot)
    sin_t, cos_t = np.sin(theta), np.cos(theta)
    K = _skew(k)
    I = np.eye(3, dtype=g_src.dtype)[None]
    R = I + sin_t[:, :, None] * K + (1.0 - cos_t)[:, :, None] * (K @ K)
    ex = np.array([1.0, 0.0, 0.0], dtype=u.dtype)[None]
    ey = np.array([0.0, 1.0, 0.0], dtype=u.dtype)[None]
    use_ex = np.abs(u[:, 0:1]) < 0.9
    basis = np.where(use_ex, ex, ey)
    axis2 = _normalize(np.cross(u, basis), 1, eps)
    K2 = _skew(axis2)
    R_anti = I + 2.0 * (K2 @ K2)
    antipar = parallel & (dot < 0.0)
    R = np.where(antipar[:, :, None], R_anti, R)
    R = np.where((parallel & (dot > 0.0))[:, :, None], I, R)
    return R.astype(np.float32)


def _split3_bf16(x):
    """x (f32) -> (x0, x1, x2) bf16 with x ~= x0 + x1 + x2 (err ~ |x| 2^-27)."""
    x = np.asarray(x, np.float32)
    x0 = x.astype(BF16)
    r1 = x - x0.astype(np.float32)
    x1 = r1.astype(BF16)
    r2 = r1 - x1.astype(np.float32)
    x2 = r2.astype(BF16)
    return x0, x1, x2


def _build_split_operands(a, bvec, cvec):
    """a: [3, N] lhs-side (already x2 scaled), bvec: [3, N] rhs-side,
    cvec: [N] the augmented (-yy or -xx) row.
    Returns lhsT [KROWS, N] bf16, rhs [KROWS, N] bf16 with
    lhsT.T @ rhs ~= a.T @ bvec + outer(1, cvec)."""
    lhs_rows = []
    rhs_rows = []
    for k in range(3):
        a0, a1, a2 = _split3_bf16(a[k])
        b0, b1, b2 = _split3_bf16(bvec[k])
        # product terms down to 2^-18 relative (drop >= 2^-27)
        for (ai, bi) in ((a0, b0), (a0, b1), (a1, b0),
                         (a0, b2), (a2, b0), (a1, b1)):
            lhs_rows.append(ai)
            rhs_rows.append(bi)
    ones = np.ones((N,), BF16)
    c0, c1, c2 = _split3_bf16(cvec)
    for ci in (c0, c1, c2):
        lhs_rows.append(ones)
        rhs_rows.append(ci)
    lhsT = np.ascontiguousarray(np.stack(lhs_rows, axis=0))
    rhs = np.ascontiguousarray(np.stack(rhs_rows, axis=0))
    assert lhsT.shape == (KROWS, N) and rhs.shape == (KROWS, N)
    return lhsT, rhs


def _decode_core(vals, segs, q_pts, c_pts, q_sq, c_sq):
    """Device window report -> exact (min_dist2[4096], argmin[4096]).

    vals/segs are [P, ROW_TILES] staging (row n = 128*rt + p at [p, rt]):
    vals = row max of G (so dev_min = q_sq - vals), segs = index of the
    WIN-wide column window containing the max. The host recomputes the WIN
    candidate distances of that window exactly (float64) and takes the
    argmin; rows whose recomputed min disagrees with the device min (fp32
    cross-window ties make the index-sum garbage) fall back to a full-row
    scan.
    """
    v1 = vals.T.reshape(N)
    sstar = np.rint(segs.T.reshape(N)).astype(np.int64)
    dev_min = q_sq - v1

    bad = (sstar < 0) | (sstar >= SEGS)
    ss = np.clip(sstar, 0, SEGS - 1)
    cand = ss[:, None] * WIN + np.arange(WIN)[None, :]      # [N, WIN]
    q64 = q_pts.astype(np.float64)                          # [3, N]
    tg = c_pts.astype(np.float64)[:, cand]                  # [3, N, WIN]
    d2 = ((q64[:, :, None] - tg) ** 2).sum(axis=0)          # [N, WIN]
    j = np.argmin(d2, axis=1)
    rows = np.arange(N)
    hmin = d2[rows, j]
    idx = cand[rows, j]
    bad |= np.abs(hmin - dev_min) > 2e-3

    if np.any(bad):
        br = np.nonzero(bad)[0]
        c64 = c_pts.astype(np.float64)
        d2f = ((q64[:, br, None] - c64[:, None, :]) ** 2).sum(axis=0)
        jf = np.argmin(d2f, axis=1)
        idx[br] = jf
        hmin[br] = d2f[np.arange(len(br)), jf]
    return hmin.astype(np.float32), idx


def _sigmoid(x):
    out = np.empty_like(x)
    pos = x >= 0
    out[pos] = 1.0 / (1.0 + np.exp(-x[pos]))
    ex = np.exp(x[~pos])
    out[~pos] = ex / (1.0 + ex)
    return out


def prepare_in_maps(src, tgt, src_n, tgt_n, g_p, k_p, g_q, k_q):
    """Host-side prep: returns the per-core device input maps."""
    R_g = _gravity_align(g_p, g_q)
    src_rot = np.einsum("bij,bjn->bin", R_g, src).astype(np.float32)
    t_center = tgt.mean(axis=2, keepdims=True) - src_rot.mean(axis=2, keepdims=True)
    s = (src_rot + t_center).astype(np.float32)  # src_init

    xx = np.sum(s * s, axis=1)  # [B, N]
    yy = np.sum(tgt * tgt, axis=1)

    in_maps = []
    for c in range(N_CORES):
        b, o = c % B, c // B
        if o == 0:
            lhsT, rhs = _build_split_operands(2.0 * s[b], tgt[b], -yy[b])
        else:
            lhsT, rhs = _build_split_operands(2.0 * tgt[b], s[b], -xx[b])
        in_maps.append({"lhsT": lhsT, "rhs": rhs})
    return in_maps


def kernel(src, tgt, src_n, tgt_n, g_p, k_p, g_q, k_q):
    src = np.asarray(src, np.float32)
    tgt = np.asarray(tgt, np.float32)
    src_n = np.asarray(src_n, np.float32)
    tgt_n = np.asarray(tgt_n, np.float32)
    g_p = np.asarray(g_p, np.float32)
    g_q = np.asarray(g_q, np.float32)
    k_p = np.asarray(k_p, np.float32)
    k_q = np.asarray(k_q, np.float32)

    R_g = _gravity_align(g_p, g_q)
    src_rot = np.einsum("bij,bjn->bin", R_g, src).astype(np.float32)
    src_n_rot = np.einsum("bij,bjn->bin", R_g, src_n).astype(np.float32)
    t_center = tgt.mean(axis=2, keepdims=True) - src_rot.mean(axis=2, keepdims=True)
    s = (src_rot + t_center).astype(np.float32)  # src_init

    xx = np.sum(s * s, axis=1)  # [B, N]
    yy = np.sum(tgt * tgt, axis=1)

    in_maps = prepare_in_maps(src, tgt, src_n, tgt_n, g_p, k_p, g_q, k_q)
    results = _get_runner()(in_maps)

    min_pq = np.empty((B, N), np.float32)
    corr_p2q = np.empty((B, N), np.int64)
    min_qp = np.empty((B, N), np.float32)
    corr_q2p = np.empty((B, N), np.int64)
    for c in range(N_CORES):
        b, o = c % B, c // B
        if o == 0:
            dmin, idx = _decode_core(results[c]["vals"], results[c]["segs"],
                                     s[b], tgt[b], xx[b], yy[b])
            min_pq[b] = dmin
            corr_p2q[b] = idx
        else:
            dmin, idx = _decode_core(results[c]["vals"], results[c]["segs"],
                                     tgt[b], s[b], yy[b], xx[b])
            min_qp[b] = dmin
            corr_q2p[b] = idx

    nn_d_p = np.sqrt(np.maximum(min_pq, 0.0) + EPS)
    nn_d_q = np.sqrt(np.maximum(min_qp, 0.0) + EPS)
    tau_p = DIST_SCALE * np.sort(nn_d_p, axis=1)[:, (N - 1) // 2][:, None]
    tau_q = DIST_SCALE * np.sort(nn_d_q, axis=1)[:, (N - 1) // 2][:, None]
    geom_p = (nn_d_p <= tau_p).astype(np.float32)
    geom_q = (nn_d_q <= tau_q).astype(np.float32)

    gq = g_q[:, :, None]
    inc_p = np.sum(src_n_rot * gq, axis=1)  # [B, N]
    inc_q = np.sum(tgt_n * gq, axis=1)
    inc_p_ref = np.take_along_axis(inc_q, corr_p2q, axis=1)
    inc_q_ref = np.take_along_axis(inc_p, corr_q2p, axis=1)

    k_eff = k_p * k_q / (k_p + k_q + EPS)  # [B,1]
    w_p = _sigmoid(CHI2_THRESH - k_eff * (inc_p - inc_p_ref) ** 2) * geom_p
    w_q = _sigmoid(CHI2_THRESH - k_eff * (inc_q - inc_q_ref) ** 2) * geom_q
    return w_p[:, None, :].astype(np.float32), w_q[:, None, :].astype(np.float32)


# revision 18
# speedup vs baseline: 1.3131x; 1.3131x over previous
"""Trainium2 kernel for nn_GravityHypothesisTester.

Heavy part (B x N x N distance matrices + row/col min/argmin) runs on 8
NeuronCores: core c handles (batch = c % 4, orientation = c // 4).
Orientation 0 reduces over tgt (rows = src points), orientation 1 reduces
over src (rows = tgt points). The PE computes G[n,m] = 2*s.t - yy[m] via an
augmented matmul so min_m dist2[n,m] = xx[n] - max_m G[n,m] and
argmin = argmax.

v2 layout (vs the fp32 baseline):
- The fp32 operands are split hi/mid/lo into bf16 triples on the host and the
  products expanded into K=21 bf16 rows (error ~2e-6, comparable to fp32
  rounding) so the PE runs at 1 cycle/row instead of fp32's 4.
- ACT copies each PSUM half-tile to SBUF; the DVE max pass then runs
  SBUF->SBUF which enables the DVE 2x_2p perf mode (2 elem/cycle on fp32).
- The argmax-recovery pass (scalar_tensor_tensor is_equal*iota, sum-accum,
  1x-only) is split between DVE and GpSimd to balance engine time.
Host does the tiny O(B*N) pre/post work (Rodrigues, means, median, sigmoid).
"""

import sys
from contextlib import ExitStack

import numpy as np
import ml_dtypes

sys.path.insert(0, "/opt/trn_rl_repo")

import concourse.bass as bass
import concourse.tile as tile
from concourse import bacc, mybir
from concourse.bass_utils import run_bass_kernel_spmd  # noqa: F401

EPS = 1e-6
CHI2_THRESH = 9.0
DIST_SCALE = 3.0
B = 4
N = 4096
P = 128
ROW_TILES = N // P          # 32
HALF = 2048                 # half-tile free size (4 PSUM banks)
MMCOLS = 512                # PSUM-bank-limited moving size (512 fp32 out)
N_CORES = 8
NHALF = ROW_TILES * 2       # 64
KROWS = 21                  # bf16-split product rows (18 coord terms + 3 yy)
BF16 = ml_dtypes.bfloat16

WIN = 64                    # candidate-window width reported to the host
SEGS = N // WIN             # 64 windows per row
SEGS_H = HALF // WIN        # 32 windows per half-tile

_NC_CACHE = {}


def _build_nc(repeat=1):
    """Build the SPMD bass program (identical on all 8 cores)."""
    key = ("nc", repeat)
    if key in _NC_CACHE:
        return _NC_CACHE[key]

    nc = bacc.Bacc("TRN2", target_bir_lowering=False)
    f32 = mybir.dt.float32
    bf16 = mybir.dt.bfloat16

    lhsT_d = nc.dram_tensor("lhsT", [KROWS, N], bf16, kind="ExternalInput")
    rhs_d = nc.dram_tensor("rhs", [KROWS, N], bf16, kind="ExternalInput")
    vals_d = nc.dram_tensor("vals", [P, ROW_TILES], f32, kind="ExternalOutput")
    segs_d = nc.dram_tensor("segs", [P, ROW_TILES], f32, kind="ExternalOutput")

    with tile.TileContext(nc) as tc, ExitStack() as ctx:
        inp = ctx.enter_context(tc.tile_pool(name="inp", bufs=1))
        stage = ctx.enter_context(tc.tile_pool(name="stage", bufs=1))
        cpool = ctx.enter_context(tc.tile_pool(name="cpool", bufs=4))
        spool = ctx.enter_context(tc.tile_pool(name="spool", bufs=3))
        psum_a = ctx.enter_context(tc.tile_pool(name="psum_a", bufs=2, space="PSUM"))

        lhsT_s = inp.tile([KROWS, N], bf16)
        rhs_s = inp.tile([KROWS, N], bf16)
        nc.sync.dma_start(lhsT_s[:], lhsT_d[:])
        nc.sync.dma_start(rhs_s[:], rhs_d[:])

        vstage = stage.tile([P, ROW_TILES], f32)
        sstage = stage.tile([P, ROW_TILES], f32)
        scr = stage.tile([P, SEGS], f32)
        iota_i = stage.tile([P, SEGS], mybir.dt.int32)
        iota_f = stage.tile([P, SEGS], f32)
        nc.gpsimd.iota(iota_i[:], pattern=[[1, SEGS]], base=0,
                       channel_multiplier=0)
        nc.vector.tensor_copy(iota_f[:], iota_i[:])

        # per-row-tile mini ops, issued one row-tile behind the segmax pass and
        # split around a segmax so the accumulator-readback stall (v1 -> s*)
        # hides under the intervening 2048-element reduce.
        def mini_v1(seg_tile, rt):
            nc.vector.tensor_scalar(
                scr[:], seg_tile[:], 0.0, None,
                op0=mybir.AluOpType.add, op1=mybir.AluOpType.max,
                accum_out=vstage[:, rt : rt + 1],
            )

        def mini_s1(seg_tile, rt):
            nc.vector.scalar_tensor_tensor(
                scr[:], seg_tile[:], vstage[:, rt : rt + 1], iota_f[:],
                op0=mybir.AluOpType.is_equal, op1=mybir.AluOpType.mult,
                accum_out=sstage[:, rt : rt + 1],
            )

        prev = None  # (seg_tile, rt) pending minis
        for t in range(ROW_TILES * repeat):
            rt = t % ROW_TILES
            seg_tile = spool.tile([P, SEGS], f32)
            for h in range(2):
                pt = psum_a.tile([P, HALF], f32)
                if t == 0 and h == 0:
                    # Dummy matmul reading only rhs_s: the PE weight-load HW
                    # slot carries a single semaphore wait, so the two input
                    # DMA waits must land on separate PE instructions.
                    nc.tensor.matmul(
                        pt[:, 0:MMCOLS], rhs_s[:, 0:P], rhs_s[:, 0:MMCOLS],
                        start=True, stop=True,
                    )
                for j in range(HALF // MMCOLS):
                    nc.tensor.matmul(
                        pt[:, bass.ts(j, MMCOLS)],
                        lhsT_s[:, bass.ts(rt, P)],
                        rhs_s[:, h * HALF + j * MMCOLS : h * HALF + (j + 1) * MMCOLS],
                        start=True,
                        stop=True,
                    )
                # ACT absorbs the PSUM read; the DVE window-max pass then runs
                # SBUF->SBUF which gets the 2x perf mode.
                sb = cpool.tile([P, HALF], f32)
                nc.scalar.copy(sb[:], pt[:])
                nc.vector.tensor_reduce(
                    seg_tile[:, h * SEGS_H : (h + 1) * SEGS_H],
                    sb[:].rearrange("p (s x) -> p s x", s=SEGS_H),
                    axis=mybir.AxisListType.X,
                    op=mybir.AluOpType.max,
                )
                if prev is not None:
                    if h == 0:
                        mini_v1(*prev)
                    else:
                        mini_s1(*prev)
            prev = (seg_tile, rt)
        mini_v1(*prev)
        mini_s1(*prev)

        nc.sync.dma_start(vals_d[:], vstage[:])
        nc.sync.dma_start(segs_d[:], sstage[:])

    nc.finalize()
    _NC_CACHE[key] = nc
    return nc


def _get_runner(repeat=1):
    """Build the sharded PJRT executable once; reuse across kernel() calls."""
    rkey = ("runner", repeat)
    if rkey in _NC_CACHE:
        return _NC_CACHE[rkey]

    import jax
    from jax.sharding import Mesh, PartitionSpec
    from jax.experimental.shard_map import shard_map
    from concourse import bass2jax

    nc = _build_nc(repeat)
    bass2jax.install_neuronx_cc_hook()

    partition_name = nc.partition_id_tensor.name if nc.partition_id_tensor else None
    in_names, out_names, out_avals, zero_outs = [], [], [], []
    for alloc in nc.m.functions[0].allocations:
        if not isinstance(alloc, mybir.MemoryLocationSet):
            continue
        name = alloc.memorylocations[0].name
        if alloc.kind == "ExternalInput":
            if name != partition_name:
                in_names.append(name)
        elif alloc.kind == "ExternalOutput":
            shape = tuple(alloc.tensor_shape)
            np_dt = mybir.dt.np(alloc.dtype)
            out_names.append(name)
            out_avals.append(jax.core.ShapedArray(shape, np_dt))
            zero_outs.append(np.zeros(shape, np_dt))

    n_params = len(in_names)
    n_outs = len(out_names)
    all_in_names = list(in_names) + list(out_names)
    if partition_name is not None:
        all_in_names.append(partition_name)
    donate = tuple(range(n_params, n_params + n_outs))

    def _body(*args):
        operands = list(args)
        if partition_name is not None:
            operands.append(bass2jax.partition_id_tensor())
        outs = bass2jax._bass_exec_p.bind(
            *operands,
            out_avals=tuple(out_avals),
            in_names=tuple(all_in_names),
            out_names=tuple(out_names),
            lowering_input_output_aliases=(),
            sim_require_finite=True,
            sim_require_nnan=True,
            nc=nc,
        )
        return tuple(outs)

    devices = jax.devices()[:N_CORES]
    mesh = Mesh(np.asarray(devices), ("core",))
    in_specs = (PartitionSpec("core"),) * (n_params + n_outs)
    out_specs = (PartitionSpec("core"),) * n_outs
    sharded = jax.jit(
        shard_map(_body, mesh=mesh, in_specs=in_specs, out_specs=out_specs,
                  check_rep=False),
        donate_argnums=donate, keep_unused=True,
    )

    def run(in_maps):
        concat_in = [
            np.concatenate([np.asarray(m[name]) for m in in_maps], axis=0)
            for name in in_names
        ]
        concat_zeros = [
            np.zeros((N_CORES * z.shape[0], *z.shape[1:]), z.dtype)
            for z in zero_outs
        ]
        out_arrs = sharded(*concat_in, *concat_zeros)
        return [
            {
                name: np.asarray(out_arrs[k]).reshape(
                    N_CORES, *out_avals[k].shape)[c]
                for k, name in enumerate(out_names)
            }
            for c in range(N_CORES)
        ]

    run.sharded = sharded
    run.mesh = mesh
    run.in_names = in_names
    run.zero_outs = zero_outs
    _NC_CACHE[rkey] = run
    return run


# ---------------- host-side numpy port of the tiny reference pieces ----------


def _normalize(x, axis, eps=EPS):
    n = np.linalg.norm(x, axis=axis, keepdims=True)
    return x / np.maximum(n, eps)


def _skew(k):
    kx, ky, kz = k[:, 0], k[:, 1], k[:, 2]
    O = np.zeros_like(kx)
    row0 = np.stack([O, -kz, ky], axis=1)
    row1 = np.stack([kz, O, -kx], axis=1)
    row2 = np.stack([-ky, kx, O], axis=1)
    return np.stack([row0, row1, row2], axis=1)


def _gravity_align(g_src, g_tgt, eps=EPS):
    u = _normalize(g_src, 1, eps)
    v = _normalize(g_tgt, 1, eps)
    axis = np.cross(u, v)
    axis_norm = np.linalg.norm(axis, axis=1, keepdims=True)
    dot = np.clip(np.sum(u * v, axis=1, keepdims=True), -1.0, 1.0)
    parallel = axis_norm < 1e-6
    k = axis / (axis_norm + eps)
    theta = np.arccos(dot)
    sin_t, cos_t = np.sin(theta), np.cos(theta)
    K = _skew(k)
    I = np.eye(3, dtype=g_src.dtype)[None]
    R = I + sin_t[:, :, None] * K + (1.0 - cos_t)[:, :, None] * (K @ K)
    ex = np.array([1.0, 0.0, 0.0], dtype=u.dtype)[None]
    ey = np.array([0.0, 1.0, 0.0], dtype=u.dtype)[None]
    use_ex = np.abs(u[:, 0:1]) < 0.9
    basis = np.where(use_ex, ex, ey)
    axis2 = _normalize(np.cross(u, basis), 1, eps)
    K2 = _skew(axis2)
    R_anti = I + 2.0 * (K2 @ K2)
    antipar = parallel & (dot < 0.0)
    R = np.where(antipar[:, :, None], R_anti, R)
    R = np.where((parallel & (dot > 0.0))[:, :, None], I, R)
    return R.astype(np.float32)


def _split3_bf16(x):
    """x (f32) -> (x0, x1, x2) bf16 with x ~= x0 + x1 + x2 (err ~ |x| 2^-27)."""
    x = np.asarray(x, np.float32)
    x0 = x.astype(BF16)
    r1 = x - x0.astype(np.float32)
    x1 = r1.astype(BF16)
    r2 = r1 - x1.astype(np.float32)
    x2 = r2.astype(BF16)
    return x0, x1, x2


def _build_split_operands(a, bvec, cvec):
    """a: [3, N] lhs-side (already x2 scaled), bvec: [3, N] rhs-side,
    cvec: [N] the augmented (-yy or -xx) row.
    Returns lhsT [KROWS, N] bf16, rhs [KROWS, N] bf16 with
    lhsT.T @ rhs ~= a.T @ bvec + outer(1, cvec)."""
    lhs_rows = []
    rhs_rows = []
    for k in range(3):
        a0, a1, a2 = _split3_bf16(a[k])
        b0, b1, b2 = _split3_bf16(bvec[k])
        # product terms down to 2^-18 relative (drop >= 2^-27)
        for (ai, bi) in ((a0, b0), (a0, b1), (a1, b0),
                         (a0, b2), (a2, b0), (a1, b1)):
            lhs_rows.append(ai)
            rhs_rows.append(bi)
    ones = np.ones((N,), BF16)
    c0, c1, c2 = _split3_bf16(cvec)
    for ci in (c0, c1, c2):
        lhs_rows.append(ones)
        rhs_rows.append(ci)
    lhsT = np.ascontiguousarray(np.stack(lhs_rows, axis=0))
    rhs = np.ascontiguousarray(np.stack(rhs_rows, axis=0))
    assert lhsT.shape == (KROWS, N) and rhs.shape == (KROWS, N)
    return lhsT, rhs


def _decode_core(vals, segs, q_pts, c_pts, q_sq, c_sq):
    """Device window report -> exact (min_dist2[4096], argmin[4096]).

    vals/segs are [P, ROW_TILES] staging (row n = 128*rt + p at [p, rt]):
    vals = row max of G (so dev_min = q_sq - vals), segs = index of the
    WIN-wide column window containing the max. The host recomputes the WIN
    candidate distances of that window exactly (float64) and takes the
    argmin; rows whose recomputed min disagrees with the device min (fp32
    cross-window ties make the index-sum garbage) fall back to a full-row
    scan.
    """
    v1 = vals.T.reshape(N)
    sstar = np.rint(segs.T.reshape(N)).astype(np.int64)
    dev_min = q_sq - v1

    bad = (sstar < 0) | (sstar >= SEGS)
    ss = np.clip(sstar, 0, SEGS - 1)
    cand = ss[:, None] * WIN + np.arange(WIN)[None, :]      # [N, WIN]
    q64 = q_pts.astype(np.float64)                          # [3, N]
    tg = c_pts.astype(np.float64)[:, cand]                  # [3, N, WIN]
    d2 = ((q64[:, :, None] - tg) ** 2).sum(axis=0)          # [N, WIN]
    j = np.argmin(d2, axis=1)
    rows = np.arange(N)
    hmin = d2[rows, j]
    idx = cand[rows, j]
    bad |= np.abs(hmin - dev_min) > 2e-3

    if np.any(bad):
        br = np.nonzero(bad)[0]
        c64 = c_pts.astype(np.float64)
        d2f = ((q64[:, br, None] - c64[:, None, :]) ** 2).sum(axis=0)
        jf = np.argmin(d2f, axis=1)
        idx[br] = jf
        hmin[br] = d2f[np.arange(len(br)), jf]
    return hmin.astype(np.float32), idx


def _sigmoid(x):
    out = np.empty_like(x)
    pos = x >= 0
    out[pos] = 1.0 / (1.0 + np.exp(-x[pos]))
    ex = np.exp(x[~pos])
    out[~pos] = ex / (1.0 + ex)
    return out


def prepare_in_maps(src, tgt, src_n, tgt_n, g_p, k_p, g_q, k_q):
    """Host-side prep: returns the per-core device input maps."""
    R_g = _gravity_align(g_p, g_q)
    src_rot = np.einsum("bij,bjn->bin", R_g, src).astype(np.float32)
    t_center = tgt.mean(axis=2, keepdims=True) - src_rot.mean(axis=2, keepdims=True)
    s = (src_rot + t_center).astype(np.float32)  # src_init

    xx = np.sum(s * s, axis=1)  # [B, N]
    yy = np.sum(tgt * tgt, axis=1)

    in_maps = []
    for c in range(N_CORES):
        b, o = c % B, c // B
        if o == 0:
            lhsT, rhs = _build_split_operands(2.0 * s[b], tgt[b], -yy[b])
        else:
            lhsT, rhs = _build_split_operands(2.0 * tgt[b], s[b], -xx[b])
        in_maps.append({"lhsT": lhsT, "rhs": rhs})
    return in_maps


def kernel(src, tgt, src_n, tgt_n, g_p, k_p, g_q, k_q):
    src = np.asarray(src, np.float32)
    tgt = np.asarray(tgt, np.float32)
    src_n = np.asarray(src_n, np.float32)
    tgt_n = np.asarray(tgt_n, np.float32)
    g_p = np.asarray(g_p, np.float32)
    g_q = np.asarray(g_q, np.float32)
    k_p = np.asarray(k_p, np.float32)
    k_q = np.asarray(k_q, np.float32)

    R_g = _gravity_align(g_p, g_q)
    src_rot = np.einsum("bij,bjn->bin", R_g, src).astype(np.float32)
    src_n_rot = np.einsum("bij,bjn->bin", R_g, src_n).astype(np.float32)
    t_center = tgt.mean(axis=2, keepdims=True) - src_rot.mean(axis=2, keepdims=True)
    s = (src_rot + t_center).astype(np.float32)  # src_init

    xx = np.sum(s * s, axis=1)  # [B, N]
    yy = np.sum(tgt * tgt, axis=1)

    in_maps = prepare_in_maps(src, tgt, src_n, tgt_n, g_p, k_p, g_q, k_q)
    results = _get_runner()(in_maps)

    min_pq = np.empty((B, N), np.float32)
    corr_p2q = np.empty((B, N), np.int64)
    min_qp = np.empty((B, N), np.float32)
    corr_q2p = np.empty((B, N), np.int64)
    for c in range(N_CORES):
        b, o = c % B, c // B
        if o == 0:
            dmin, idx = _decode_core(results[c]["vals"], results[c]["segs"],
                                     s[b], tgt[b], xx[b], yy[b])
            min_pq[b] = dmin
            corr_p2q[b] = idx
        else:
            dmin, idx = _decode_core(results[c]["vals"], results[c]["segs"],
                                     tgt[b], s[b], yy[b], xx[b])
            min_qp[b] = dmin
            corr_q2p[b] = idx

    nn_d_p = np.sqrt(np.maximum(min_pq, 0.0) + EPS)
    nn_d_q = np.sqrt(np.maximum(min_qp, 0.0) + EPS)
    tau_p = DIST_SCALE * np.sort(nn_d_p, axis=1)[:, (N - 1) // 2][:, None]
    tau_q = DIST_SCALE * np.sort(nn_d_q, axis=1)[:, (N - 1) // 2][:, None]
    geom_p = (nn_d_p <= tau_p).astype(np.float32)
    geom_q = (nn_d_q <= tau_q).astype(np.float32)

    gq = g_q[:, :, None]
    inc_p = np.sum(src_n_rot * gq, axis=1)  # [B, N]
    inc_q = np.sum(tgt_n * gq, axis=1)
    inc_p_ref = np.take_along_axis(inc_q, corr_p2q, axis=1)
    inc_q_ref = np.take_along_axis(inc_p, corr_q2p, axis=1)

    k_eff = k_p * k_q / (k_p + k_q + EPS)  # [B,1]
    w_p = _sigmoid(CHI2_THRESH - k_eff * (inc_p - inc_p_ref) ** 2) * geom_p
    w_q = _sigmoid(CHI2_THRESH - k_eff * (inc_q - inc_q_ref) ** 2) * geom_q
    return w_p[:, None, :].astype(np.float32), w_q[:, None, :].astype(np.float32)
